# revision 49
# baseline (speedup 1.0000x reference)
"""Multi-head causal self-attention (B=4, N=2048, D=1024, H=16) on 8 TRN2 cores.

Sharding: 8 cores = 4 batches x 2 head-groups (8 heads / 512 dims each).

v2 design (single flowing pipeline per strip, strip = 128 q-dims = 2 heads):
  - S^T tiles for the two heads of a strip are emitted as ADJACENT 64x128
    row-tiled matmuls (tile_position (0,0) / (64,0) auto-derived from the
    AP base partitions) so they execute concurrently on the PE array:
    2x throughput on the K=64-contraction score matmuls.
  - exp on the Scalar engine is the attention-phase metronome (~1 elem/
    lane/cycle @1.2GHz); all other PE work (next strip's Q/K projections,
    V projection, previous strip's O-projection chunk) is interleaved as
    "filler" items between attention units so the PE never idles and the
    HAM clock gate stays warm.
  - Softmax denominators come from a ones-column appended to V (row HD of
    the PV accumulator); normalization deferred per strip; the reciprocal
    is computed with a single custom-DVE op (reciprocal_approx_fast), so
    the Scalar engine runs Exp only -> exactly one ACT table load.
  - O-projection: out = sum_c attnT_c^T @ Wo_c accumulated in SBUF (bf16)
    chunk-by-chunk as each strip's attnT gets normalized; the final strip
    is normalized per-query-strip so the output DMA drains progressively.

PSUM budget (8 banks): 4 = score tiles (2 per head, ping-pong across
units), 2 = PV accumulators (one per head), 2 = shared Q/K/V/O scratch.

Dtypes: all matmul inputs bf16, PSUM fp32, attnT/probs bf16, O accumulator
and DMA-out bf16 (host sums the two group partials in fp32 and adds bo).
"""

import collections

import numpy as np
import ml_dtypes

import concourse.bass as bass
import concourse.tile as tile
from concourse import bacc, mybir
from concourse import bass_utils
from concourse._compat import with_exitstack
from concourse.bass import ts, ds

B, N, D, H, HD = 4, 2048, 1024, 16, 64
GROUPS = 2              # head groups (cores per batch)
DC = D // GROUPS        # 512 dims per core
HPC = H // GROUPS       # 8 heads per core
P = 128
QW = 512                # query strip width / matmul free dim
NDIN = D // P           # 8 contraction chunks for QKV
NSTRIP = DC // P        # 4 dq strips per core (2 heads each)
NTT = N // P            # 16 token tiles
NTS = N // QW           # 4 token strips
NQB = QW // P           # 4 query blocks per strip

F32 = mybir.dt.float32
BF16 = mybir.dt.bfloat16

SERIAL = False          # debug: drain fillers eagerly instead of interleaving
SEQ_HEADS = False       # debug: emit the two heads' S matmuls sequentially
DEBUG_DUMP = False      # debug: dump attnT/vplus/qts/kts to DRAM outputs
CARRY = False           # carry PV+evac of the last unit across qs boundaries
                        # (disabled: triggers a data race, see CARRY notes)
FLUSH_STRIP_END = True  # with CARRY: still flush at strip boundaries


def _emit(ctx, tc, xT, wq, wk, wv, wo, bq, bk, bv, masks, out, dbg=None):
    nc = tc.nc
    EXP = mybir.ActivationFunctionType.Exp

    const = ctx.enter_context(tc.tile_pool(name="const", bufs=1))
    p_xt = ctx.enter_context(tc.tile_pool(name="p_xt", bufs=1))
    p_st = ctx.enter_context(tc.tile_pool(name="p_st", bufs=1, space="PSUM"))
    p_pv = ctx.enter_context(tc.tile_pool(name="p_pv", bufs=1, space="PSUM"))
    p_mm = ctx.enter_context(tc.tile_pool(name="p_mm", bufs=2, space="PSUM"))
    p_pt = ctx.enter_context(tc.tile_pool(name="p_pt", bufs=6))
    p_qk = ctx.enter_context(tc.tile_pool(name="p_qk", bufs=2))
    p_w = ctx.enter_context(tc.tile_pool(name="p_w", bufs=2))
    p_small = ctx.enter_context(tc.tile_pool(name="p_small", bufs=2))
    p_dram = ctx.enter_context(tc.tile_pool(name="p_dram", bufs=2, space="DRAM"))

    wqr = wq.rearrange("(c p) f -> p c f", p=P)
    wkr = wk.rearrange("(c p) f -> p c f", p=P)
    wvr = wv.rearrange("(c p) f -> p c f", p=P)
    wor = wo.rearrange("(c p) f -> p c f", p=P)
    xTr = xT.rearrange("(c p) n -> p c n", p=P)

    # x^T resident; batched strided DMAs, strip-major so the first Q/K
    # matmuls can start after ~2 transfers
    xt = p_xt.tile([P, NDIN, N], BF16)
    for t in range(NTS):
        for ch in range(2):
            nc.sync.dma_start(
                out=xt[:, ds(4 * ch, 4), ts(t, QW)],
                in_=xTr[:, ds(4 * ch, 4), ts(t, QW)])

    # strip-0 Q/K weights lead the SWDGE queue (first matmuls need them)
    def load_qk_weights(s):
        wqs = p_w.tile([P, NDIN, P], BF16, tag="wq", name="wqs")
        wks = p_w.tile([P, NDIN, P], BF16, tag="wk", name="wks")
        nc.gpsimd.dma_start(out=wqs, in_=wqr[:, :, ts(s, P)])
        nc.gpsimd.dma_start(out=wks, in_=wkr[:, :, ts(s, P)])
        return wqs, wks

    wqs0, wks0 = load_qk_weights(0)

    # ---- constants (SWDGE queue, after the strip-0 weights) ----
    maskt = const.tile([P, P], BF16)
    nc.gpsimd.dma_start(out=maskt, in_=masks)
    # doubled mask for batched diagonal-region multiplies
    maskt2 = const.tile([P, 2, P], BF16)
    nc.gpsimd.dma_start(out=maskt2[:, 0, :], in_=masks)
    nc.gpsimd.dma_start(out=maskt2[:, 1, :], in_=masks)
    bqt = const.tile([P, NSTRIP], F32)
    nc.gpsimd.dma_start(out=bqt, in_=bq.rearrange("(s p) -> p s", p=P))
    bkt = const.tile([P, NSTRIP], F32)
    nc.gpsimd.dma_start(out=bkt, in_=bk.rearrange("(s p) -> p s", p=P))
    bvb = const.tile([P, DC], F32)
    nc.gpsimd.dma_start(out=bvb, in_=bv.unsqueeze(0).partition_broadcast(P))

    # V projection weights, then Wo (needed latest)
    wvt = const.tile([P, NDIN, DC], BF16)
    nc.gpsimd.dma_start(out=wvt, in_=wvr)

    # persistent per-batch tensors
    attnT = const.tile([P, NSTRIP, N], BF16)                # attn^T (normalized in place)
    vplus = const.tile([P, NTT, HPC, HD + 1], BF16)         # V | ones column
    osb = const.tile([P, NTT, D], BF16)                     # O accumulator (partial out)
    ones_f32 = const.tile([P, NTT * HPC], F32)
    nc.vector.memset(ones_f32, 1.0)
    nc.vector.tensor_copy(
        out=vplus[:, :, :, HD:HD + 1],
        in_=ones_f32.rearrange("p (a b) -> p a b", b=HPC).unsqueeze(3),
    )
    # bf16 ones rows: lhsT of the K=1 broadcast matmuls in normalization
    onesb = const.tile([P, P], BF16)
    nc.vector.tensor_copy(out=onesb, in_=ones_f32[:, 0:P])

    # warm the PE clock gate (HAM) during the DMA fill: a burst of cheap
    # matmuls keeps the 2.4GHz clock up before the first projections arrive
    for w in range(40):
        pswarm = p_mm.tile([P, P], F32, tag="mm", name="pswarm")
        nc.tensor.matmul(pswarm, lhsT=onesb, rhs=onesb, start=True, stop=True)

    wot = const.tile([P, NSTRIP, D], BF16)
    nc.gpsimd.dma_start(out=wot, in_=wor)



    # ---- filler machinery: small units of PE work pumped between ----
    # ---- attention units so the PE never head-of-line blocks      ----
    filler_q = collections.deque()
    pending = [None]    # carried PV+evac of the previous attention unit

    def pump(n):
        if SERIAL:
            n = 10000
        for _ in range(n):
            if not filler_q:
                return
            filler_q.popleft()()

    def drain():
        while filler_q:
            filler_q.popleft()()

    # ---- QKV projection work ----
    def qk_items(s, wqs, wks, qts, kts, t_lo=0):
        """Q^T/K^T for strip s as a list of filler items (2 per psum tile)."""
        items = []
        for t in range(t_lo, NTS):
            for wt, dst, bias in ((wqs, qts, bqt), (wks, kts, bkt)):
                box = {}

                def a(wt=wt, t=t, box=box):
                    ps = p_mm.tile([P, QW], F32, tag="mm", name="psqk")
                    for c in range(4):
                        nc.tensor.matmul(
                            ps, lhsT=wt[:, c, :], rhs=xt[:, c, ts(t, QW)],
                            start=(c == 0), stop=False)
                    box["ps"] = ps

                def b(wt=wt, dst=dst, bias=bias, s=s, t=t, box=box):
                    ps = box.pop("ps")
                    for c in range(4, NDIN):
                        nc.tensor.matmul(
                            ps, lhsT=wt[:, c, :], rhs=xt[:, c, ts(t, QW)],
                            start=False, stop=(c == NDIN - 1))
                    nc.vector.tensor_scalar_add(
                        out=dst[:, ts(t, QW)], in0=ps, scalar1=bias[:, s:s + 1])

                items.append(a)
                items.append(b)
        return items

    def v_items(tt_lo, tt_hi):
        """V = x @ Wv + bv for token tiles [tt_lo, tt_hi) as filler items."""
        items = []
        for tt in range(tt_lo, tt_hi):
            box = {}

            def a(tt=tt, box=box):
                ps = p_mm.tile([P, DC], F32, tag="mm", name="psv")
                for c in range(4):
                    nc.tensor.matmul(
                        ps, lhsT=xt[:, c, ts(tt, P)], rhs=wvt[:, c, :],
                        start=(c == 0), stop=False)
                box["ps"] = ps

            def b(tt=tt, box=box):
                ps = box.pop("ps")
                for c in range(4, NDIN):
                    nc.tensor.matmul(
                        ps, lhsT=xt[:, c, ts(tt, P)], rhs=wvt[:, c, :],
                        start=False, stop=(c == NDIN - 1))
                nc.vector.tensor_add(
                    out=vplus[:, tt, :, 0:HD],
                    in0=ps.rearrange("p (h d) -> p h d", d=HD),
                    in1=bvb.rearrange("p (h d) -> p h d", d=HD))

            items.append(a)
            items.append(b)
        return items

    # ---- O-projection: osb[tt] = sum_c attnT_c^T @ Wo_c ----
    # chunk 0 lands during strip 1 (copy), chunk 1 during strip 2 (add),
    # chunks 2+3 as one PSUM-accumulated pair in the per-qs tail (norm3) —
    # spreads the DVE evacuation load evenly across strips.
    def o_single_items(c):
        items = []
        for tt in range(NTT):
            for half in range(2):
                def f(c=c, tt=tt, half=half):
                    pso = p_mm.tile([P, QW], F32, tag="mm", name="pso")
                    nc.tensor.matmul(
                        pso, lhsT=attnT[:, c, ts(tt, P)],
                        rhs=wot[:, c, ds(half * QW, QW)],
                        start=True, stop=True)
                    dst = osb[:, tt, ds(half * QW, QW)]
                    if c == 0:
                        nc.vector.tensor_copy(out=dst, in_=pso)
                    else:
                        nc.vector.tensor_add(out=dst, in0=pso, in1=dst)
                items.append(f)
        return items

    # ---- normalization: 1/den via custom-DVE reciprocal, broadcast across
    # ---- partitions with a K=1 ones-matmul (all on-chip, no DRAM trip) ----
    def recip_bf16(sums_sb):
        recip_sb = p_small.tile([P, 2, QW], F32, tag="recip", name="recip_sb")
        nc.vector.reciprocal_approx_fast(out=recip_sb, in_=sums_sb)
        recip_bf = p_small.tile([P, 2, QW], BF16, tag="recipb", name="recip_bf")
        nc.vector.tensor_copy(out=recip_bf, in_=recip_sb)
        return recip_bf

    def norm_qs(s, qs, recip_bf):
        r0 = 32 * qs
        for h2 in range(2):
            po = h2 * HD
            rb = p_mm.tile([P, QW], F32, tag="mm", name="rb")
            nc.tensor.matmul(
                rb, lhsT=onesb[r0:r0 + 1, :],
                rhs=recip_bf[r0:r0 + 1, h2, :], start=True, stop=True,
                tile_position=(r0, 0))
            sl = attnT[po:po + HD, s, ts(qs, QW)]
            nc.vector.tensor_mul(out=sl, in0=sl, in1=rb[po:po + HD, :])

    def norm_items(s, sums_sb):
        items = []
        box = {}

        def rcp(sums_sb=sums_sb, box=box):
            box["rb"] = recip_bf16(sums_sb)

        items.append(rcp)
        for qs in range(NTS):
            items.append(lambda s=s, qs=qs, box=box: norm_qs(s, qs, box["rb"]))
        return items

    # ---- per-qs normalization + O chunk + out-DMA for the LAST strip ----
    def norm3_items(s, qs, sums_sb):
        box = {}

        def rcp(sums_sb=sums_sb, box=box):
            box["rb"] = recip_bf16(sums_sb)

        def fin(s=s, qs=qs, box=box):
            norm_qs(s, qs, box["rb"])
            for tt in range(4 * qs, 4 * qs + 4):
                for half in range(2):
                    pso = p_mm.tile([P, QW], F32, tag="mm", name="pso3")
                    for c in (2, 3):
                        nc.tensor.matmul(
                            pso, lhsT=attnT[:, c, ts(tt, P)],
                            rhs=wot[:, c, ds(half * QW, QW)],
                            start=(c == 2), stop=(c == 3))
                    dst = osb[:, tt, ds(half * QW, QW)]
                    nc.vector.tensor_add(out=dst, in0=pso, in1=dst)
                nc.sync.dma_start(out=out[ts(tt, P), :], in_=osb[:, tt, :])

        return [rcp, fin]

    # ---- attention: strip s = heads (2s, 2s+1) ----
    def attn_strip(s, qts, kts, sums_sb):
        """Emit attention for both heads of strip s, unit-by-unit.

        A unit covers 2 key blocks for BOTH heads: 4 row-tiled S^T matmuls
        (T0/T8 interleaved -> concurrent), 2 exp activations, then (with
        one unit of lookahead) 4 PV matmuls. diagA/diagB replicate the
        baseline's packed shrinking-width diagonal handling per head.
        """
        hs = (2 * s, 2 * s + 1)

        def emit_s(qs, unit):
            kind, ip = unit
            nfull = NQB * qs
            q0 = qs * QW
            psts = []
            pts = []
            if kind == "full":
                for h2 in range(2):
                    psts.append(p_st.tile(
                        [P, 2, QW], F32, tag=f"st{h2}", name=f"pst{h2}"))
                order = ([(h2, j2) for h2 in range(2) for j2 in range(2)]
                         if SEQ_HEADS else
                         [(h2, j2) for j2 in range(2) for h2 in range(2)])
                for h2, j2 in order:
                    kc = 2 * ip + j2
                    po = h2 * HD
                    nc.tensor.matmul(
                        psts[h2][:, j2, :],
                        lhsT=kts[po:po + HD, ts(kc, P)],
                        rhs=qts[po:po + HD, ts(qs, QW)],
                        start=True, stop=True)
                for h2 in range(2):
                    pt = p_pt.tile([P, 2, QW], BF16, tag="pt", name="pt")
                    nc.scalar.activation(out=pt, in_=psts[h2], func=EXP, scale=0.125)
                    pts.append(pt)
                return pts
            if kind == "diagA":
                # j=0: kc=nfull,   queries [0:512), tri mask on cols 0:128
                # j=1: kc=nfull+1, queries [128:512), tri mask on cols 0:128
                for h2 in range(2):
                    psts.append(p_st.tile(
                        [P, 2, QW], F32, tag=f"st{h2}", name=f"pst{h2}"))
                for h2 in range(2):
                    po = h2 * HD
                    nc.tensor.matmul(
                        psts[h2][:, 0, :],
                        lhsT=kts[po:po + HD, ts(nfull, P)],
                        rhs=qts[po:po + HD, ts(qs, QW)],
                        start=True, stop=True)
                for h2 in range(2):
                    po = h2 * HD
                    nc.tensor.matmul(
                        psts[h2][:, 1, 0:3 * P],
                        lhsT=kts[po:po + HD, ts(nfull + 1, P)],
                        rhs=qts[po:po + HD, ds(q0 + P, 3 * P)],
                        start=True, stop=True)
                for h2 in range(2):
                    pt = p_pt.tile([P, 2, QW], BF16, tag="pt", name="pt")
                    nc.scalar.activation(out=pt, in_=psts[h2], func=EXP, scale=0.125)
                    nc.vector.tensor_mul(pt[:, :, 0:P], pt[:, :, 0:P], maskt2)
                    pts.append(pt)
                return pts
            # diagB: j=2: kc=nfull+2, queries [256:512) at cols 0:256;
            #        j=3: kc=nfull+3, queries [384:512) at cols 256:384
            for h2 in range(2):
                psts.append(p_st.tile([P, QW], F32, tag=f"st{h2}", name=f"pst{h2}"))
            for h2 in range(2):
                po = h2 * HD
                nc.tensor.matmul(
                    psts[h2][:, 0:2 * P],
                    lhsT=kts[po:po + HD, ts(nfull + 2, P)],
                    rhs=qts[po:po + HD, ds(q0 + 2 * P, 2 * P)],
                    start=True, stop=True)
            for h2 in range(2):
                po = h2 * HD
                nc.tensor.matmul(
                    psts[h2][:, 2 * P:3 * P],
                    lhsT=kts[po:po + HD, ts(nfull + 3, P)],
                    rhs=qts[po:po + HD, ds(q0 + 3 * P, P)],
                    start=True, stop=True)
            for h2 in range(2):
                pt = p_pt.tile([P, QW], BF16, tag="pt", name="pt")
                nc.scalar.activation(
                    out=pt[:, 0:3 * P], in_=psts[h2][:, 0:3 * P],
                    func=EXP, scale=0.125)
                # masked regions are cols 0:P and 2P:3P -> one strided mul
                ptr = pt.rearrange("p (a b) -> p a b", b=2 * P)
                nc.vector.tensor_mul(ptr[:, :, 0:P], ptr[:, :, 0:P], maskt2)
                pts.append(pt)
            return pts

        def emit_pv(qs, unit, pts, pvps):
            kind, ip = unit
            nfull = NQB * qs
            if kind == "full":
                for j2 in range(2):
                    kc = 2 * ip + j2
                    for h2 in range(2):
                        nc.tensor.matmul(
                            pvps[h2], lhsT=vplus[:, kc, hs[h2], :],
                            rhs=pts[h2][:, j2, :],
                            start=(kc == 0), stop=False)
            elif kind == "diagA":
                for h2 in range(2):
                    nc.tensor.matmul(
                        pvps[h2], lhsT=vplus[:, nfull, hs[h2], :],
                        rhs=pts[h2][:, 0, :],
                        start=(nfull == 0), stop=False)
                for h2 in range(2):
                    nc.tensor.matmul(
                        pvps[h2][:, P:4 * P], lhsT=vplus[:, nfull + 1, hs[h2], :],
                        rhs=pts[h2][:, 1, 0:3 * P], start=False, stop=False)
            else:
                for h2 in range(2):
                    nc.tensor.matmul(
                        pvps[h2][:, 2 * P:4 * P],
                        lhsT=vplus[:, nfull + 2, hs[h2], :],
                        rhs=pts[h2][:, 0:2 * P], start=False, stop=False)
                for h2 in range(2):
                    nc.tensor.matmul(
                        pvps[h2][:, 3 * P:4 * P],
                        lhsT=vplus[:, nfull + 3, hs[h2], :],
                        rhs=pts[h2][:, 2 * P:3 * P], start=False, stop=True)

        def evac(qs, pvps):
            for h2 in range(2):
                po = h2 * HD
                nc.vector.tensor_copy(
                    out=sums_sb[32 * qs:32 * qs + 1, h2, :],
                    in_=pvps[h2][HD:HD + 1, :])
                # keep evacs off the Scalar queue: an ACT-side copy here
                # head-of-line blocks the next qs's exp (the metronome)
                nc.vector.tensor_copy(
                    out=attnT[po:po + HD, s, ts(qs, QW)],
                    in_=pvps[h2][0:HD, :])
            if s == NSTRIP - 1:
                filler_q.extend(norm3_items(s, qs, sums_sb))

        # flat unit list across qs; the one-unit PV lookahead and the qs
        # eviction run inside the NEXT unit's window (carried across qs and
        # strip boundaries via `pending`) so the exp metronome never stalls
        pump_n = 4 if s == 0 else 2
        pvps = None
        for qs in range(NTS):
            units = [("full", ip) for ip in range(NQB * qs // 2)]
            units.append(("diagA", None))
            units.append(("diagB", None))
            last = len(units) - 1
            for iu, u in enumerate(units):
                pts = emit_s(qs, u)
                if pending[0] is not None:
                    pending[0]()
                    pending[0] = None
                if iu == 0:
                    pvps = [
                        p_pv.tile([HD + 1, QW], F32, tag=f"pv{h2}",
                                  name=f"pvp{h2}")
                        for h2 in range(2)
                    ]
                if iu == last:
                    def fl(qs=qs, u=u, pts=pts, pvps=pvps):
                        emit_pv(qs, u, pts, pvps)
                        evac(qs, pvps)
                    pending[0] = fl
                else:
                    pending[0] = (lambda qs=qs, u=u, pts=pts, pvps=pvps:
                                  emit_pv(qs, u, pts, pvps))
                if not CARRY and iu == last:
                    # flush the PV+evac immediately (no fillers in between)
                    # so the next qs's S matmuls follow as soon as possible
                    pending[0]()
                    pending[0] = None
                    pump(pump_n + 2)
                elif CARRY and iu == last:
                    pass  # hold fillers while the boundary PV is pending
                else:
                    pump(pump_n)
        if CARRY and FLUSH_STRIP_END and pending[0] is not None:
            pending[0]()
            pending[0] = None

    # ================= main schedule =================
    # upfront: only what strip-0 qs=0 needs (Q/K token strip 0, V tiles 0-3);
    # everything else becomes filler work inside the attention stream
    qts = {}
    kts = {}
    sums = {}
    qts[0] = p_qk.tile([P, N], BF16, tag="qt", name="qts")
    kts[0] = p_qk.tile([P, N], BF16, tag="kt", name="kts")
    for f in qk_items(0, wqs0, wks0, qts[0], kts[0])[:4]:
        f()
    for f in v_items(0, 4):
        f()

    for s in range(NSTRIP):
        sums[s] = p_small.tile([P, 2, QW], F32, tag="sums", name="sums_sb")
        nc.gpsimd.memset(sums[s], 1.0)
        if s == 0:
            # rest of strip-0 Q/K, then V jit (one qs ahead of first use)
            filler_q.extend(qk_items(0, wqs0, wks0, qts[0], kts[0], t_lo=1))
            filler_q.extend(v_items(4, 8))
            filler_q.extend(v_items(8, 12))
            filler_q.extend(v_items(12, 16))
        if s + 1 < NSTRIP:
            wqs, wks = load_qk_weights(s + 1)
            qts[s + 1] = p_qk.tile([P, N], BF16, tag="qt", name="qts")
            kts[s + 1] = p_qk.tile([P, N], BF16, tag="kt", name="kts")
            filler_q.extend(qk_items(s + 1, wqs, wks, qts[s + 1], kts[s + 1]))
        attn_strip(s, qts[s], kts[s], sums[s])
        if s < NSTRIP - 1:
            # normalization for this strip runs as fillers inside the next
            # strip's attention
            filler_q.extend(norm_items(s, sums[s]))
        if s < 2:
            filler_q.extend(o_single_items(s))   # O chunk s during strip s+1
    if pending[0] is not None:
        pending[0]()
        pending[0] = None
    drain()
    if dbg is not None:
        nc.sync.dma_start(out=dbg["attnT"], in_=attnT)
        nc.sync.dma_start(out=dbg["vplus"], in_=vplus.rearrange("p a h d -> p (a h d)"))
        for s in range(NSTRIP):
            nc.sync.dma_start(out=dbg["qts"].rearrange("(s p) n -> s p n", p=P)[s],
                              in_=qts[s])
            nc.sync.dma_start(out=dbg["kts"].rearrange("(s p) n -> s p n", p=P)[s],
                              in_=kts[s])


_emit_wrapped = with_exitstack(_emit)

_NC_CACHE = None


def _build():
    global _NC_CACHE
    if _NC_CACHE is not None:
        return _NC_CACHE
    nc = bacc.Bacc("TRN2", target_bir_lowering=False, debug=False)
    xT = nc.dram_tensor("xt", [D, N], BF16, kind="ExternalInput").ap()
    wq = nc.dram_tensor("wq", [D, DC], BF16, kind="ExternalInput").ap()
    wk = nc.dram_tensor("wk", [D, DC], BF16, kind="ExternalInput").ap()
    wv = nc.dram_tensor("wv", [D, DC], BF16, kind="ExternalInput").ap()
    wo = nc.dram_tensor("wo", [DC, D], BF16, kind="ExternalInput").ap()
    bq = nc.dram_tensor("bq", [DC], F32, kind="ExternalInput").ap()
    bk = nc.dram_tensor("bk", [DC], F32, kind="ExternalInput").ap()
    bv = nc.dram_tensor("bv", [DC], F32, kind="ExternalInput").ap()
    masks = nc.dram_tensor("masks", [P, P], BF16, kind="ExternalInput").ap()
    out = nc.dram_tensor("out", [N, D], BF16, kind="ExternalOutput").ap()
    dbg = None
    if DEBUG_DUMP:
        dbg = {
            "attnT": nc.dram_tensor(
                "dbg_attnT", [P, NSTRIP, N], BF16, kind="ExternalOutput").ap(),
            "vplus": nc.dram_tensor(
                "dbg_vplus", [P, NTT * HPC * (HD + 1)], BF16,
                kind="ExternalOutput").ap(),
            "qts": nc.dram_tensor(
                "dbg_qts", [NSTRIP * P, N], BF16, kind="ExternalOutput").ap(),
            "kts": nc.dram_tensor(
                "dbg_kts", [NSTRIP * P, N], BF16, kind="ExternalOutput").ap(),
        }
    with tile.TileContext(nc) as tc:
        _emit_wrapped(tc, xT, wq, wk, wv, wo, bq, bk, bv, masks, out, dbg=dbg)
    nc.compile()
    _NC_CACHE = nc
    return nc


def _make_masks():
    # triangular 0/1 tile for the diagonal blocks of S^T: key <= query kept
    return np.triu(np.ones((P, P), np.float32)).astype(ml_dtypes.bfloat16)


def _in_maps(x, Wq, bq, Wk, bk, Wv, bv, Wo):
    masks = _make_masks()
    maps = []
    for b in range(B):
        xt_b = np.ascontiguousarray(np.asarray(x[b]).T)
        for g in range(GROUPS):
            sl = slice(g * DC, (g + 1) * DC)
            bf = ml_dtypes.bfloat16
            maps.append({
                "xt": xt_b.astype(bf),
                "wq": np.ascontiguousarray(Wq[:, sl]).astype(bf),
                "wk": np.ascontiguousarray(Wk[:, sl]).astype(bf),
                "wv": np.ascontiguousarray(Wv[:, sl]).astype(bf),
                "wo": np.ascontiguousarray(Wo[sl, :]).astype(bf),
                "bq": np.ascontiguousarray(bq[sl]),
                "bk": np.ascontiguousarray(bk[sl]),
                "bv": np.ascontiguousarray(bv[sl]),
                "masks": masks,
            })
    return maps


def run(inputs, trace=False, tmpdir=None):
    """Build+run on 8 cores. Returns (out [B,N,D] f32, BassKernelResults)."""
    x = np.asarray(inputs["x"], np.float32)
    args = [np.asarray(inputs[k], np.float32) for k in
            ("Wq", "bq", "Wk", "bk", "Wv", "bv", "Wo")]
    bo = np.asarray(inputs["bo"], np.float32)
    nc = _build()
    maps = _in_maps(x, *args)
    if trace:
        bass_utils.upload_artifacts = lambda d: d
    res = bass_utils.run_bass_kernel_spmd(
        nc, maps, core_ids=list(range(8)), trace=trace, tmpdir=tmpdir)
    out = np.empty((B, N, D), np.float32)
    for b in range(B):
        out[b] = (res.results[2 * b]["out"].astype(np.float32)
                  + res.results[2 * b + 1]["out"].astype(np.float32) + bo)
    return out, res


def kernel(**inputs):
    out, _ = run(inputs)
    return out


# revision 50
# speedup vs baseline: 1.1551x; 1.1551x over previous
"""Multi-head causal self-attention (B=4, N=2048, D=1024, H=16) on 8 TRN2 cores.

Sharding: 8 cores = 4 batches x 2 head-groups (8 heads / 512 dims each).

v2 design (single flowing pipeline per strip, strip = 128 q-dims = 2 heads):
  - S^T tiles for the two heads of a strip are emitted as ADJACENT 64x128
    row-tiled matmuls (tile_position (0,0) / (64,0) auto-derived from the
    AP base partitions) so they execute concurrently on the PE array:
    2x throughput on the K=64-contraction score matmuls.
  - exp on the Scalar engine is the attention-phase metronome (~1 elem/
    lane/cycle @1.2GHz); all other PE work (next strip's Q/K projections,
    V projection, previous strip's O-projection chunk) is interleaved as
    "filler" items between attention units so the PE never idles and the
    HAM clock gate stays warm.
  - Softmax denominators come from a ones-column appended to V (row HD of
    the PV accumulator); normalization deferred per strip; the reciprocal
    is computed with a single custom-DVE op (reciprocal_approx_fast), so
    the Scalar engine runs Exp only -> exactly one ACT table load.
  - O-projection: out = sum_c attnT_c^T @ Wo_c accumulated in SBUF (bf16)
    chunk-by-chunk as each strip's attnT gets normalized; the final strip
    is normalized per-query-strip so the output DMA drains progressively.

PSUM budget (8 banks): 4 = score tiles (2 per head, ping-pong across
units), 2 = PV accumulators (one per head), 2 = shared Q/K/V/O scratch.

Dtypes: all matmul inputs bf16, PSUM fp32, attnT/probs bf16, O accumulator
and DMA-out bf16 (host sums the two group partials in fp32 and adds bo).
"""

import collections

import numpy as np
import ml_dtypes

import concourse.bass as bass
import concourse.tile as tile
from concourse import bacc, mybir
from concourse import bass_utils
from concourse._compat import with_exitstack
from concourse.bass import ts, ds

B, N, D, H, HD = 4, 2048, 1024, 16, 64
GROUPS = 2              # head groups (cores per batch)
DC = D // GROUPS        # 512 dims per core
HPC = H // GROUPS       # 8 heads per core
P = 128
QW = 512                # query strip width / matmul free dim
NDIN = D // P           # 8 contraction chunks for QKV
NSTRIP = DC // P        # 4 dq strips per core (2 heads each)
NTT = N // P            # 16 token tiles
NTS = N // QW           # 4 token strips
NQB = QW // P           # 4 query blocks per strip

F32 = mybir.dt.float32
BF16 = mybir.dt.bfloat16

SERIAL = False          # debug: drain fillers eagerly instead of interleaving
SEQ_HEADS = False       # debug: emit the two heads' S matmuls sequentially
DEBUG_DUMP = False      # debug: dump attnT/vplus/qts/kts to DRAM outputs
CARRY = False           # carry PV+evac of the last unit across qs boundaries
                        # (disabled: triggers a data race, see CARRY notes)
FLUSH_STRIP_END = True  # with CARRY: still flush at strip boundaries


def _emit(ctx, tc, xT, wq, wk, wv, wo, bq, bk, bv, masks, out, dbg=None):
    nc = tc.nc
    EXP = mybir.ActivationFunctionType.Exp

    const = ctx.enter_context(tc.tile_pool(name="const", bufs=1))
    p_xt = ctx.enter_context(tc.tile_pool(name="p_xt", bufs=1))
    p_st = ctx.enter_context(tc.tile_pool(name="p_st", bufs=1, space="PSUM"))
    p_pv = ctx.enter_context(tc.tile_pool(name="p_pv", bufs=1, space="PSUM"))
    p_mm = ctx.enter_context(tc.tile_pool(name="p_mm", bufs=2, space="PSUM"))
    p_pt = ctx.enter_context(tc.tile_pool(name="p_pt", bufs=6))
    p_qk = ctx.enter_context(tc.tile_pool(name="p_qk", bufs=2))
    p_w = ctx.enter_context(tc.tile_pool(name="p_w", bufs=2))
    p_small = ctx.enter_context(tc.tile_pool(name="p_small", bufs=2))
    p_dram = ctx.enter_context(tc.tile_pool(name="p_dram", bufs=2, space="DRAM"))

    wqr = wq.rearrange("(c p) f -> p c f", p=P)
    wkr = wk.rearrange("(c p) f -> p c f", p=P)
    wvr = wv.rearrange("(c p) f -> p c f", p=P)
    wor = wo.rearrange("(c p) f -> p c f", p=P)
    xTr = xT.rearrange("(c p) n -> p c n", p=P)

    # x^T resident; batched strided DMAs, strip-major so the first Q/K
    # matmuls can start after ~2 transfers
    xt = p_xt.tile([P, NDIN, N], BF16)
    for t in range(NTS):
        for ch in range(2):
            nc.sync.dma_start(
                out=xt[:, ds(4 * ch, 4), ts(t, QW)],
                in_=xTr[:, ds(4 * ch, 4), ts(t, QW)])

    # strip-0 Q/K weights lead the SWDGE queue (first matmuls need them)
    def load_qk_weights(s):
        wqs = p_w.tile([P, NDIN, P], BF16, tag="wq", name="wqs")
        wks = p_w.tile([P, NDIN, P], BF16, tag="wk", name="wks")
        nc.gpsimd.dma_start(out=wqs, in_=wqr[:, :, ts(s, P)])
        nc.gpsimd.dma_start(out=wks, in_=wkr[:, :, ts(s, P)])
        return wqs, wks

    wqs0, wks0 = load_qk_weights(0)

    # ---- constants (SWDGE queue, after the strip-0 weights) ----
    maskt = const.tile([P, P], BF16)
    nc.gpsimd.dma_start(out=maskt, in_=masks)
    # doubled mask for batched diagonal-region multiplies
    maskt2 = const.tile([P, 2, P], BF16)
    nc.gpsimd.dma_start(out=maskt2[:, 0, :], in_=masks)
    nc.gpsimd.dma_start(out=maskt2[:, 1, :], in_=masks)
    bqt = const.tile([P, NSTRIP], F32)
    nc.gpsimd.dma_start(out=bqt, in_=bq.rearrange("(s p) -> p s", p=P))
    bkt = const.tile([P, NSTRIP], F32)
    nc.gpsimd.dma_start(out=bkt, in_=bk.rearrange("(s p) -> p s", p=P))
    bvb = const.tile([P, DC], F32)
    nc.gpsimd.dma_start(out=bvb, in_=bv.unsqueeze(0).partition_broadcast(P))

    # V projection weights, then Wo (needed latest)
    wvt = const.tile([P, NDIN, DC], BF16)
    nc.gpsimd.dma_start(out=wvt, in_=wvr)

    # persistent per-batch tensors
    attnT = const.tile([P, NSTRIP, N], BF16)                # attn^T (normalized in place)
    vplus = const.tile([P, NTT, HPC, HD + 1], BF16)         # V | ones column
    osb = const.tile([P, NTT, D], BF16)                     # O accumulator (partial out)
    ones_f32 = const.tile([P, NTT * HPC], F32)
    nc.vector.memset(ones_f32, 1.0)
    nc.vector.tensor_copy(
        out=vplus[:, :, :, HD:HD + 1],
        in_=ones_f32.rearrange("p (a b) -> p a b", b=HPC).unsqueeze(3),
    )
    # bf16 ones rows: lhsT of the K=1 broadcast matmuls in normalization
    onesb = const.tile([P, P], BF16)
    nc.vector.tensor_copy(out=onesb, in_=ones_f32[:, 0:P])

    # warm the PE clock gate (HAM) during the DMA fill: a burst of cheap
    # accumulating matmuls keeps the 2.4GHz clock up before the first
    # projections arrive
    pswarm = p_mm.tile([P, P], F32, tag="mm", name="pswarm")
    for w in range(40):
        nc.tensor.matmul(pswarm, lhsT=onesb, rhs=onesb,
                         start=(w == 0), stop=(w == 39))

    wot = const.tile([P, NSTRIP, D], BF16)
    nc.gpsimd.dma_start(out=wot, in_=wor)



    # ---- filler machinery: small units of PE work pumped between ----
    # ---- attention units so the PE never head-of-line blocks      ----
    filler_q = collections.deque()
    pending = [None]    # carried PV+evac of the previous attention unit

    def pump(n):
        if SERIAL:
            n = 10000
        for _ in range(n):
            if not filler_q:
                return
            filler_q.popleft()()

    def drain():
        while filler_q:
            filler_q.popleft()()

    # ---- QKV projection work ----
    def qk_items(s, wqs, wks, qts, kts, t_lo=0):
        """Q^T/K^T for strip s as a list of filler items (2 per psum tile)."""
        items = []
        for t in range(t_lo, NTS):
            for wt, dst, bias in ((wqs, qts, bqt), (wks, kts, bkt)):
                box = {}

                def a(wt=wt, t=t, box=box):
                    ps = p_mm.tile([P, QW], F32, tag="mm", name="psqk")
                    for c in range(4):
                        nc.tensor.matmul(
                            ps, lhsT=wt[:, c, :], rhs=xt[:, c, ts(t, QW)],
                            start=(c == 0), stop=False)
                    box["ps"] = ps

                def b(wt=wt, dst=dst, bias=bias, s=s, t=t, box=box):
                    ps = box.pop("ps")
                    for c in range(4, NDIN):
                        nc.tensor.matmul(
                            ps, lhsT=wt[:, c, :], rhs=xt[:, c, ts(t, QW)],
                            start=False, stop=(c == NDIN - 1))
                    nc.vector.tensor_scalar_add(
                        out=dst[:, ts(t, QW)], in0=ps, scalar1=bias[:, s:s + 1])

                items.append(a)
                items.append(b)
        return items

    def v_items(tt_lo, tt_hi):
        """V = x @ Wv + bv for token tiles [tt_lo, tt_hi) as filler items."""
        items = []
        for tt in range(tt_lo, tt_hi):
            box = {}

            def a(tt=tt, box=box):
                ps = p_mm.tile([P, DC], F32, tag="mm", name="psv")
                for c in range(4):
                    nc.tensor.matmul(
                        ps, lhsT=xt[:, c, ts(tt, P)], rhs=wvt[:, c, :],
                        start=(c == 0), stop=False)
                box["ps"] = ps

            def b(tt=tt, box=box):
                ps = box.pop("ps")
                for c in range(4, NDIN):
                    nc.tensor.matmul(
                        ps, lhsT=xt[:, c, ts(tt, P)], rhs=wvt[:, c, :],
                        start=False, stop=(c == NDIN - 1))
                nc.vector.tensor_add(
                    out=vplus[:, tt, :, 0:HD],
                    in0=ps.rearrange("p (h d) -> p h d", d=HD),
                    in1=bvb.rearrange("p (h d) -> p h d", d=HD))

            items.append(a)
            items.append(b)
        return items

    # ---- O-projection: osb[tt] = sum_c attnT_c^T @ Wo_c ----
    # chunk 0 lands during strip 1 (copy), chunk 1 during strip 2 (add),
    # chunks 2+3 as one PSUM-accumulated pair in the per-qs tail (norm3) —
    # spreads the DVE evacuation load evenly across strips.
    def o_single_items(c):
        items = []
        for tt in range(NTT):
            for half in range(2):
                def f(c=c, tt=tt, half=half):
                    pso = p_mm.tile([P, QW], F32, tag="mm", name="pso")
                    nc.tensor.matmul(
                        pso, lhsT=attnT[:, c, ts(tt, P)],
                        rhs=wot[:, c, ds(half * QW, QW)],
                        start=True, stop=True)
                    dst = osb[:, tt, ds(half * QW, QW)]
                    if c == 0:
                        nc.vector.tensor_copy(out=dst, in_=pso)
                    else:
                        nc.vector.tensor_add(out=dst, in0=pso, in1=dst)
                items.append(f)
        return items

    # ---- normalization: 1/den via custom-DVE reciprocal, broadcast across
    # ---- partitions with a K=1 ones-matmul (all on-chip, no DRAM trip) ----
    def recip_bf16(sums_sb):
        recip_sb = p_small.tile([P, 2, QW], F32, tag="recip", name="recip_sb")
        nc.vector.reciprocal_approx_fast(out=recip_sb, in_=sums_sb)
        recip_bf = p_small.tile([P, 2, QW], BF16, tag="recipb", name="recip_bf")
        nc.vector.tensor_copy(out=recip_bf, in_=recip_sb)
        return recip_bf

    def norm_qs(s, qs, recip_bf):
        r0 = 32 * qs
        for h2 in range(2):
            po = h2 * HD
            rb = p_mm.tile([P, QW], F32, tag="mm", name="rb")
            nc.tensor.matmul(
                rb, lhsT=onesb[r0:r0 + 1, :],
                rhs=recip_bf[r0:r0 + 1, h2, :], start=True, stop=True,
                tile_position=(r0, 0))
            sl = attnT[po:po + HD, s, ts(qs, QW)]
            nc.vector.tensor_mul(out=sl, in0=sl, in1=rb[po:po + HD, :])

    def norm_items(s, sums_sb):
        items = []
        box = {}

        def rcp(sums_sb=sums_sb, box=box):
            box["rb"] = recip_bf16(sums_sb)

        items.append(rcp)
        for qs in range(NTS):
            items.append(lambda s=s, qs=qs, box=box: norm_qs(s, qs, box["rb"]))
        return items

    # ---- per-qs normalization + O chunk + out-DMA for the LAST strip ----
    def norm3_items(s, qs, sums_sb):
        box = {}

        def rcp(sums_sb=sums_sb, box=box):
            box["rb"] = recip_bf16(sums_sb)

        def fin(s=s, qs=qs, box=box):
            norm_qs(s, qs, box["rb"])
            for tt in range(4 * qs, 4 * qs + 4):
                for half in range(2):
                    pso = p_mm.tile([P, QW], F32, tag="mm", name="pso3")
                    for c in (2, 3):
                        nc.tensor.matmul(
                            pso, lhsT=attnT[:, c, ts(tt, P)],
                            rhs=wot[:, c, ds(half * QW, QW)],
                            start=(c == 2), stop=(c == 3))
                    dst = osb[:, tt, ds(half * QW, QW)]
                    nc.vector.tensor_add(out=dst, in0=pso, in1=dst)
                nc.sync.dma_start(out=out[ts(tt, P), :], in_=osb[:, tt, :])

        return [rcp, fin]

    # ---- attention: strip s = heads (2s, 2s+1) ----
    def attn_strip(s, qts, kts, sums_sb):
        """Emit attention for both heads of strip s, unit-by-unit.

        A unit covers 2 key blocks for BOTH heads: 4 row-tiled S^T matmuls
        (T0/T8 interleaved -> concurrent), 2 exp activations, then (with
        one unit of lookahead) 4 PV matmuls. diagA/diagB replicate the
        baseline's packed shrinking-width diagonal handling per head.
        """
        hs = (2 * s, 2 * s + 1)

        def emit_s(qs, unit):
            kind, ip = unit
            nfull = NQB * qs
            q0 = qs * QW
            psts = []
            pts = []
            if kind == "full":
                for h2 in range(2):
                    psts.append(p_st.tile(
                        [P, 2, QW], F32, tag=f"st{h2}", name=f"pst{h2}"))
                order = ([(h2, j2) for h2 in range(2) for j2 in range(2)]
                         if SEQ_HEADS else
                         [(h2, j2) for j2 in range(2) for h2 in range(2)])
                for h2, j2 in order:
                    kc = 2 * ip + j2
                    po = h2 * HD
                    nc.tensor.matmul(
                        psts[h2][:, j2, :],
                        lhsT=kts[po:po + HD, ts(kc, P)],
                        rhs=qts[po:po + HD, ts(qs, QW)],
                        start=True, stop=True)
                for h2 in range(2):
                    pt = p_pt.tile([P, 2, QW], BF16, tag="pt", name="pt")
                    nc.scalar.activation(out=pt, in_=psts[h2], func=EXP, scale=0.125)
                    pts.append(pt)
                return pts
            if kind == "diagA":
                # j=0: kc=nfull,   queries [0:512), tri mask on cols 0:128
                # j=1: kc=nfull+1, queries [128:512), tri mask on cols 0:128
                for h2 in range(2):
                    psts.append(p_st.tile(
                        [P, 2, QW], F32, tag=f"st{h2}", name=f"pst{h2}"))
                for h2 in range(2):
                    po = h2 * HD
                    nc.tensor.matmul(
                        psts[h2][:, 0, :],
                        lhsT=kts[po:po + HD, ts(nfull, P)],
                        rhs=qts[po:po + HD, ts(qs, QW)],
                        start=True, stop=True)
                for h2 in range(2):
                    po = h2 * HD
                    nc.tensor.matmul(
                        psts[h2][:, 1, 0:3 * P],
                        lhsT=kts[po:po + HD, ts(nfull + 1, P)],
                        rhs=qts[po:po + HD, ds(q0 + P, 3 * P)],
                        start=True, stop=True)
                for h2 in range(2):
                    pt = p_pt.tile([P, 2, QW], BF16, tag="pt", name="pt")
                    nc.scalar.activation(out=pt, in_=psts[h2], func=EXP, scale=0.125)
                    nc.vector.tensor_mul(pt[:, :, 0:P], pt[:, :, 0:P], maskt2)
                    pts.append(pt)
                return pts
            # diagB: j=2: kc=nfull+2, queries [256:512) at cols 0:256;
            #        j=3: kc=nfull+3, queries [384:512) at cols 256:384
            for h2 in range(2):
                psts.append(p_st.tile([P, QW], F32, tag=f"st{h2}", name=f"pst{h2}"))
            for h2 in range(2):
                po = h2 * HD
                nc.tensor.matmul(
                    psts[h2][:, 0:2 * P],
                    lhsT=kts[po:po + HD, ts(nfull + 2, P)],
                    rhs=qts[po:po + HD, ds(q0 + 2 * P, 2 * P)],
                    start=True, stop=True)
            for h2 in range(2):
                po = h2 * HD
                nc.tensor.matmul(
                    psts[h2][:, 2 * P:3 * P],
                    lhsT=kts[po:po + HD, ts(nfull + 3, P)],
                    rhs=qts[po:po + HD, ds(q0 + 3 * P, P)],
                    start=True, stop=True)
            for h2 in range(2):
                pt = p_pt.tile([P, QW], BF16, tag="pt", name="pt")
                nc.scalar.activation(
                    out=pt[:, 0:3 * P], in_=psts[h2][:, 0:3 * P],
                    func=EXP, scale=0.125)
                # masked regions are cols 0:P and 2P:3P -> one strided mul
                ptr = pt.rearrange("p (a b) -> p a b", b=2 * P)
                nc.vector.tensor_mul(ptr[:, :, 0:P], ptr[:, :, 0:P], maskt2)
                pts.append(pt)
            return pts

        def emit_pv(qs, unit, pts, pvps):
            kind, ip = unit
            nfull = NQB * qs
            if kind == "full":
                for j2 in range(2):
                    kc = 2 * ip + j2
                    for h2 in range(2):
                        nc.tensor.matmul(
                            pvps[h2], lhsT=vplus[:, kc, hs[h2], :],
                            rhs=pts[h2][:, j2, :],
                            start=(kc == 0), stop=False)
            elif kind == "diagA":
                for h2 in range(2):
                    nc.tensor.matmul(
                        pvps[h2], lhsT=vplus[:, nfull, hs[h2], :],
                        rhs=pts[h2][:, 0, :],
                        start=(nfull == 0), stop=False)
                for h2 in range(2):
                    nc.tensor.matmul(
                        pvps[h2][:, P:4 * P], lhsT=vplus[:, nfull + 1, hs[h2], :],
                        rhs=pts[h2][:, 1, 0:3 * P], start=False, stop=False)
            else:
                for h2 in range(2):
                    nc.tensor.matmul(
                        pvps[h2][:, 2 * P:4 * P],
                        lhsT=vplus[:, nfull + 2, hs[h2], :],
                        rhs=pts[h2][:, 0:2 * P], start=False, stop=False)
                for h2 in range(2):
                    nc.tensor.matmul(
                        pvps[h2][:, 3 * P:4 * P],
                        lhsT=vplus[:, nfull + 3, hs[h2], :],
                        rhs=pts[h2][:, 2 * P:3 * P], start=False, stop=True)

        def evac(qs, pvps):
            for h2 in range(2):
                po = h2 * HD
                nc.vector.tensor_copy(
                    out=sums_sb[32 * qs:32 * qs + 1, h2, :],
                    in_=pvps[h2][HD:HD + 1, :])
                # keep evacs off the Scalar queue: an ACT-side copy here
                # head-of-line blocks the next qs's exp (the metronome)
                nc.vector.tensor_copy(
                    out=attnT[po:po + HD, s, ts(qs, QW)],
                    in_=pvps[h2][0:HD, :])
            if s == NSTRIP - 1:
                filler_q.extend(norm3_items(s, qs, sums_sb))

        # flat unit list across qs; the one-unit PV lookahead and the qs
        # eviction run inside the NEXT unit's window (carried across qs and
        # strip boundaries via `pending`) so the exp metronome never stalls
        pump_n = 4 if s == 0 else 2
        pvps = None
        for qs in range(NTS):
            units = [("full", ip) for ip in range(NQB * qs // 2)]
            units.append(("diagA", None))
            units.append(("diagB", None))
            last = len(units) - 1
            for iu, u in enumerate(units):
                pts = emit_s(qs, u)
                if pending[0] is not None:
                    pending[0]()
                    pending[0] = None
                if iu == 0:
                    pvps = [
                        p_pv.tile([HD + 1, QW], F32, tag=f"pv{h2}",
                                  name=f"pvp{h2}")
                        for h2 in range(2)
                    ]
                if iu == last:
                    def fl(qs=qs, u=u, pts=pts, pvps=pvps):
                        emit_pv(qs, u, pts, pvps)
                        evac(qs, pvps)
                    pending[0] = fl
                else:
                    pending[0] = (lambda qs=qs, u=u, pts=pts, pvps=pvps:
                                  emit_pv(qs, u, pts, pvps))
                if not CARRY and iu == last:
                    # flush the PV+evac immediately (no fillers in between)
                    # so the next qs's S matmuls follow as soon as possible
                    pending[0]()
                    pending[0] = None
                    pump(pump_n + 2)
                elif CARRY and iu == last:
                    pass  # hold fillers while the boundary PV is pending
                else:
                    pump(pump_n)
        if CARRY and FLUSH_STRIP_END and pending[0] is not None:
            pending[0]()
            pending[0] = None

    # ================= main schedule =================
    # upfront: only what strip-0 qs=0 needs (Q/K token strip 0, V tiles 0-3);
    # everything else becomes filler work inside the attention stream
    qts = {}
    kts = {}
    sums = {}
    qts[0] = p_qk.tile([P, N], BF16, tag="qt", name="qts")
    kts[0] = p_qk.tile([P, N], BF16, tag="kt", name="kts")
    for f in qk_items(0, wqs0, wks0, qts[0], kts[0])[:4]:
        f()
    for f in v_items(0, 4):
        f()

    for s in range(NSTRIP):
        sums[s] = p_small.tile([P, 2, QW], F32, tag="sums", name="sums_sb")
        nc.gpsimd.memset(sums[s], 1.0)
        if s == 0:
            # rest of strip-0 Q/K, then V jit (one qs ahead of first use)
            filler_q.extend(qk_items(0, wqs0, wks0, qts[0], kts[0], t_lo=1))
            filler_q.extend(v_items(4, 8))
            filler_q.extend(v_items(8, 12))
            filler_q.extend(v_items(12, 16))
        if s + 1 < NSTRIP:
            wqs, wks = load_qk_weights(s + 1)
            qts[s + 1] = p_qk.tile([P, N], BF16, tag="qt", name="qts")
            kts[s + 1] = p_qk.tile([P, N], BF16, tag="kt", name="kts")
            filler_q.extend(qk_items(s + 1, wqs, wks, qts[s + 1], kts[s + 1]))
        attn_strip(s, qts[s], kts[s], sums[s])
        if s < NSTRIP - 1:
            # normalization for this strip runs as fillers inside the next
            # strip's attention
            filler_q.extend(norm_items(s, sums[s]))
        if s < 2:
            filler_q.extend(o_single_items(s))   # O chunk s during strip s+1
    if pending[0] is not None:
        pending[0]()
        pending[0] = None
    drain()
    if dbg is not None:
        nc.sync.dma_start(out=dbg["attnT"], in_=attnT)
        nc.sync.dma_start(out=dbg["vplus"], in_=vplus.rearrange("p a h d -> p (a h d)"))
        for s in range(NSTRIP):
            nc.sync.dma_start(out=dbg["qts"].rearrange("(s p) n -> s p n", p=P)[s],
                              in_=qts[s])
            nc.sync.dma_start(out=dbg["kts"].rearrange("(s p) n -> s p n", p=P)[s],
                              in_=kts[s])


_emit_wrapped = with_exitstack(_emit)

_NC_CACHE = None


def _build():
    global _NC_CACHE
    if _NC_CACHE is not None:
        return _NC_CACHE
    nc = bacc.Bacc("TRN2", target_bir_lowering=False, debug=False)
    xT = nc.dram_tensor("xt", [D, N], BF16, kind="ExternalInput").ap()
    wq = nc.dram_tensor("wq", [D, DC], BF16, kind="ExternalInput").ap()
    wk = nc.dram_tensor("wk", [D, DC], BF16, kind="ExternalInput").ap()
    wv = nc.dram_tensor("wv", [D, DC], BF16, kind="ExternalInput").ap()
    wo = nc.dram_tensor("wo", [DC, D], BF16, kind="ExternalInput").ap()
    bq = nc.dram_tensor("bq", [DC], F32, kind="ExternalInput").ap()
    bk = nc.dram_tensor("bk", [DC], F32, kind="ExternalInput").ap()
    bv = nc.dram_tensor("bv", [DC], F32, kind="ExternalInput").ap()
    masks = nc.dram_tensor("masks", [P, P], BF16, kind="ExternalInput").ap()
    out = nc.dram_tensor("out", [N, D], BF16, kind="ExternalOutput").ap()
    dbg = None
    if DEBUG_DUMP:
        dbg = {
            "attnT": nc.dram_tensor(
                "dbg_attnT", [P, NSTRIP, N], BF16, kind="ExternalOutput").ap(),
            "vplus": nc.dram_tensor(
                "dbg_vplus", [P, NTT * HPC * (HD + 1)], BF16,
                kind="ExternalOutput").ap(),
            "qts": nc.dram_tensor(
                "dbg_qts", [NSTRIP * P, N], BF16, kind="ExternalOutput").ap(),
            "kts": nc.dram_tensor(
                "dbg_kts", [NSTRIP * P, N], BF16, kind="ExternalOutput").ap(),
        }
    with tile.TileContext(nc) as tc:
        _emit_wrapped(tc, xT, wq, wk, wv, wo, bq, bk, bv, masks, out, dbg=dbg)
    nc.compile()
    _NC_CACHE = nc
    return nc


def _make_masks():
    # triangular 0/1 tile for the diagonal blocks of S^T: key <= query kept
    return np.triu(np.ones((P, P), np.float32)).astype(ml_dtypes.bfloat16)


def _in_maps(x, Wq, bq, Wk, bk, Wv, bv, Wo):
    masks = _make_masks()
    maps = []
    for b in range(B):
        xt_b = np.ascontiguousarray(np.asarray(x[b]).T)
        for g in range(GROUPS):
            sl = slice(g * DC, (g + 1) * DC)
            bf = ml_dtypes.bfloat16
            maps.append({
                "xt": xt_b.astype(bf),
                "wq": np.ascontiguousarray(Wq[:, sl]).astype(bf),
                "wk": np.ascontiguousarray(Wk[:, sl]).astype(bf),
                "wv": np.ascontiguousarray(Wv[:, sl]).astype(bf),
                "wo": np.ascontiguousarray(Wo[sl, :]).astype(bf),
                "bq": np.ascontiguousarray(bq[sl]),
                "bk": np.ascontiguousarray(bk[sl]),
                "bv": np.ascontiguousarray(bv[sl]),
                "masks": masks,
            })
    return maps


def run(inputs, trace=False, tmpdir=None):
    """Build+run on 8 cores. Returns (out [B,N,D] f32, BassKernelResults)."""
    x = np.asarray(inputs["x"], np.float32)
    args = [np.asarray(inputs[k], np.float32) for k in
            ("Wq", "bq", "Wk", "bk", "Wv", "bv", "Wo")]
    bo = np.asarray(inputs["bo"], np.float32)
    nc = _build()
    maps = _in_maps(x, *args)
    if trace:
        bass_utils.upload_artifacts = lambda d: d
    res = bass_utils.run_bass_kernel_spmd(
        nc, maps, core_ids=list(range(8)), trace=trace, tmpdir=tmpdir)
    out = np.empty((B, N, D), np.float32)
    for b in range(B):
        out[b] = (res.results[2 * b]["out"].astype(np.float32)
                  + res.results[2 * b + 1]["out"].astype(np.float32) + bo)
    return out, res


def kernel(**inputs):
    out, _ = run(inputs)
    return out


# revision 52
# speedup vs baseline: 1.2269x; 1.0622x over previous
"""Multi-head causal self-attention (B=4, N=2048, D=1024, H=16) on 8 TRN2 cores.

Sharding: 8 cores = 4 batches x 2 head-groups (8 heads / 512 dims each).

v2 design (single flowing pipeline per strip, strip = 128 q-dims = 2 heads):
  - S^T tiles for the two heads of a strip are emitted as ADJACENT 64x128
    row-tiled matmuls (tile_position (0,0) / (64,0) auto-derived from the
    AP base partitions) so they execute concurrently on the PE array:
    2x throughput on the K=64-contraction score matmuls.
  - exp on the Scalar engine is the attention-phase metronome (~1 elem/
    lane/cycle @1.2GHz); all other PE work (next strip's Q/K projections,
    V projection, previous strip's O-projection chunk) is interleaved as
    "filler" items between attention units so the PE never idles and the
    HAM clock gate stays warm.
  - Softmax denominators come from a ones-column appended to V (row HD of
    the PV accumulator); normalization deferred per strip; the reciprocal
    is computed with a single custom-DVE op (reciprocal_approx_fast), so
    the Scalar engine runs Exp only -> exactly one ACT table load.
  - O-projection: out = sum_c attnT_c^T @ Wo_c accumulated in SBUF (bf16)
    chunk-by-chunk as each strip's attnT gets normalized; the final strip
    is normalized per-query-strip so the output DMA drains progressively.

PSUM budget (8 banks): 4 = score tiles (2 per head, ping-pong across
units), 2 = PV accumulators (one per head), 2 = shared Q/K/V/O scratch.

Dtypes: all matmul inputs bf16, PSUM fp32, attnT/probs bf16, O accumulator
and DMA-out bf16 (host sums the two group partials in fp32 and adds bo).
"""

import collections

import numpy as np
import ml_dtypes

import concourse.bass as bass
import concourse.tile as tile
from concourse import bacc, mybir
from concourse import bass_utils
from concourse._compat import with_exitstack
from concourse.bass import ts, ds

B, N, D, H, HD = 4, 2048, 1024, 16, 64
GROUPS = 2              # head groups (cores per batch)
DC = D // GROUPS        # 512 dims per core
HPC = H // GROUPS       # 8 heads per core
P = 128
QW = 512                # query strip width / matmul free dim
NDIN = D // P           # 8 contraction chunks for QKV
NSTRIP = DC // P        # 4 dq strips per core (2 heads each)
NTT = N // P            # 16 token tiles
NTS = N // QW           # 4 token strips
NQB = QW // P           # 4 query blocks per strip

F32 = mybir.dt.float32
BF16 = mybir.dt.bfloat16

SERIAL = False          # debug: drain fillers eagerly instead of interleaving
SEQ_HEADS = False       # debug: emit the two heads' S matmuls sequentially
DEBUG_DUMP = False      # debug: dump attnT/vplus/qts/kts to DRAM outputs
CARRY = False           # carry PV+evac of the last unit across qs boundaries
                        # (disabled: triggers a data race, see CARRY notes)
FLUSH_STRIP_END = True  # with CARRY: still flush at strip boundaries


def _emit(ctx, tc, xT, wq, wk, wv, wo, bq, bk, bv, masks, out, dbg=None):
    nc = tc.nc
    EXP = mybir.ActivationFunctionType.Exp

    const = ctx.enter_context(tc.tile_pool(name="const", bufs=1))
    p_xt = ctx.enter_context(tc.tile_pool(name="p_xt", bufs=1))
    p_st = ctx.enter_context(tc.tile_pool(name="p_st", bufs=1, space="PSUM"))
    p_pv = ctx.enter_context(tc.tile_pool(name="p_pv", bufs=1, space="PSUM"))
    p_mm = ctx.enter_context(tc.tile_pool(name="p_mm", bufs=2, space="PSUM"))
    p_pt = ctx.enter_context(tc.tile_pool(name="p_pt", bufs=4))
    p_qk = ctx.enter_context(tc.tile_pool(name="p_qk", bufs=2))
    p_w = ctx.enter_context(tc.tile_pool(name="p_w", bufs=2))
    p_small = ctx.enter_context(tc.tile_pool(name="p_small", bufs=2))
    p_dram = ctx.enter_context(tc.tile_pool(name="p_dram", bufs=2, space="DRAM"))

    wqr = wq.rearrange("(c p) f -> p c f", p=P)
    wkr = wk.rearrange("(c p) f -> p c f", p=P)
    wvr = wv.rearrange("(c p) f -> p c f", p=P)
    wor = wo.rearrange("(c p) f -> p c f", p=P)
    xTr = xT.rearrange("(c p) n -> p c n", p=P)

    # x^T resident; batched strided DMAs, strip-major so the first Q/K
    # matmuls can start after ~2 transfers
    xt = p_xt.tile([P, NDIN, N], BF16)
    for t in range(NTS):
        for ch in range(2):
            nc.sync.dma_start(
                out=xt[:, ds(4 * ch, 4), ts(t, QW)],
                in_=xTr[:, ds(4 * ch, 4), ts(t, QW)])

    # strip-0 Q/K weights lead the SWDGE queue (first matmuls need them)
    def load_qk_weights(s):
        wqs = p_w.tile([P, NDIN, P], BF16, tag="wq", name="wqs")
        wks = p_w.tile([P, NDIN, P], BF16, tag="wk", name="wks")
        nc.gpsimd.dma_start(out=wqs, in_=wqr[:, :, ts(s, P)])
        nc.gpsimd.dma_start(out=wks, in_=wkr[:, :, ts(s, P)])
        return wqs, wks

    wqs0, wks0 = load_qk_weights(0)

    # ---- constants (SWDGE queue, after the strip-0 weights) ----
    maskt = const.tile([P, P], BF16)
    nc.gpsimd.dma_start(out=maskt, in_=masks)
    # doubled mask for batched diagonal-region multiplies
    maskt2 = const.tile([P, 2, P], BF16)
    nc.gpsimd.dma_start(out=maskt2[:, 0, :], in_=masks)
    nc.gpsimd.dma_start(out=maskt2[:, 1, :], in_=masks)
    bqt = const.tile([P, NSTRIP], F32)
    nc.gpsimd.dma_start(out=bqt, in_=bq.rearrange("(s p) -> p s", p=P))
    bkt = const.tile([P, NSTRIP], F32)
    nc.gpsimd.dma_start(out=bkt, in_=bk.rearrange("(s p) -> p s", p=P))
    bvb = const.tile([P, DC], F32)
    nc.gpsimd.dma_start(out=bvb, in_=bv.unsqueeze(0).partition_broadcast(P))

    # V projection weights, then Wo (needed latest)
    wvt = const.tile([P, NDIN, DC], BF16)
    nc.gpsimd.dma_start(out=wvt, in_=wvr)

    # persistent per-batch tensors
    attnT = const.tile([P, NSTRIP, N], BF16)                # attn^T (normalized in place)
    vplus = const.tile([P, NTT, HPC, HD + 1], BF16)         # V | ones column
    osb = const.tile([P, NTT, D], BF16)                     # O accumulator (partial out)
    ones_f32 = const.tile([P, NTT * HPC], F32)
    nc.vector.memset(ones_f32, 1.0)
    nc.vector.tensor_copy(
        out=vplus[:, :, :, HD:HD + 1],
        in_=ones_f32.rearrange("p (a b) -> p a b", b=HPC).unsqueeze(3),
    )
    # bf16 ones rows: lhsT of the K=1 broadcast matmuls in normalization
    onesb = const.tile([P, P], BF16)
    nc.vector.tensor_copy(out=onesb, in_=ones_f32[:, 0:P])

    # warm the PE clock gate (HAM) during the DMA fill: a burst of cheap
    # accumulating matmuls keeps the 2.4GHz clock up before the first
    # projections arrive
    pswarm = p_mm.tile([P, P], F32, tag="mm", name="pswarm")
    for w in range(40):
        nc.tensor.matmul(pswarm, lhsT=onesb, rhs=onesb,
                         start=(w == 0), stop=(w == 39))

    wot = const.tile([P, NSTRIP, D], BF16)
    nc.gpsimd.dma_start(out=wot, in_=wor)



    # ---- filler machinery: small units of PE work pumped between ----
    # ---- attention units so the PE never head-of-line blocks      ----
    filler_q = collections.deque()
    pending = [None]    # carried PV+evac of the previous attention unit

    def pump(n):
        if SERIAL:
            n = 10000
        for _ in range(n):
            if not filler_q:
                return
            filler_q.popleft()()

    def drain():
        while filler_q:
            filler_q.popleft()()

    # ---- QKV projection work ----
    def qk_items(s, wqs, wks, qts, kts, t_lo=0):
        """Q^T/K^T for strip s as a list of filler items (2 per psum tile)."""
        items = []
        for t in range(t_lo, NTS):
            for wt, dst, bias in ((wqs, qts, bqt), (wks, kts, bkt)):
                box = {}

                def a(wt=wt, t=t, box=box):
                    ps = p_mm.tile([P, QW], F32, tag="mm", name="psqk")
                    for c in range(4):
                        nc.tensor.matmul(
                            ps, lhsT=wt[:, c, :], rhs=xt[:, c, ts(t, QW)],
                            start=(c == 0), stop=False)
                    box["ps"] = ps

                def b(wt=wt, dst=dst, bias=bias, s=s, t=t, box=box):
                    ps = box.pop("ps")
                    for c in range(4, NDIN):
                        nc.tensor.matmul(
                            ps, lhsT=wt[:, c, :], rhs=xt[:, c, ts(t, QW)],
                            start=False, stop=(c == NDIN - 1))
                    nc.vector.tensor_scalar_add(
                        out=dst[:, ts(t, QW)], in0=ps, scalar1=bias[:, s:s + 1])

                items.append(a)
                items.append(b)
        return items

    def v_items(tt_lo, tt_hi):
        """V = x @ Wv + bv for token tiles [tt_lo, tt_hi) as filler items."""
        items = []
        for tt in range(tt_lo, tt_hi):
            box = {}

            def a(tt=tt, box=box):
                ps = p_mm.tile([P, DC], F32, tag="mm", name="psv")
                for c in range(4):
                    nc.tensor.matmul(
                        ps, lhsT=xt[:, c, ts(tt, P)], rhs=wvt[:, c, :],
                        start=(c == 0), stop=False)
                box["ps"] = ps

            def b(tt=tt, box=box):
                ps = box.pop("ps")
                for c in range(4, NDIN):
                    nc.tensor.matmul(
                        ps, lhsT=xt[:, c, ts(tt, P)], rhs=wvt[:, c, :],
                        start=False, stop=(c == NDIN - 1))
                nc.vector.tensor_add(
                    out=vplus[:, tt, :, 0:HD],
                    in0=ps.rearrange("p (h d) -> p h d", d=HD),
                    in1=bvb.rearrange("p (h d) -> p h d", d=HD))

            items.append(a)
            items.append(b)
        return items

    # ---- O-projection: osb[tt] = sum_c attnT_c^T @ Wo_c ----
    # chunk 0 lands during strip 1 (copy), chunk 1 during strip 2 (add),
    # chunks 2+3 as one PSUM-accumulated pair in the per-qs tail (norm3) —
    # spreads the DVE evacuation load evenly across strips.
    def o_single_items(c):
        items = []
        for tt in range(NTT):
            for half in range(2):
                def f(c=c, tt=tt, half=half):
                    pso = p_mm.tile([P, QW], F32, tag="mm", name="pso")
                    nc.tensor.matmul(
                        pso, lhsT=attnT[:, c, ts(tt, P)],
                        rhs=wot[:, c, ds(half * QW, QW)],
                        start=True, stop=True)
                    dst = osb[:, tt, ds(half * QW, QW)]
                    if c == 0:
                        nc.vector.tensor_copy(out=dst, in_=pso)
                    else:
                        nc.vector.tensor_add(out=dst, in0=pso, in1=dst)
                items.append(f)
        return items

    # ---- normalization: 1/den via custom-DVE reciprocal, broadcast across
    # ---- partitions with a K=1 ones-matmul (all on-chip, no DRAM trip) ----
    def recip_bf16(sums_sb):
        recip_sb = p_small.tile([P, 2, QW], F32, tag="recip", name="recip_sb")
        nc.vector.reciprocal_approx_fast(out=recip_sb, in_=sums_sb)
        recip_bf = p_small.tile([P, 2, QW], BF16, tag="recipb", name="recip_bf")
        nc.vector.tensor_copy(out=recip_bf, in_=recip_sb)
        return recip_bf

    def norm_qs(s, qs, recip_bf):
        r0 = 32 * qs
        for h2 in range(2):
            po = h2 * HD
            rb = p_mm.tile([P, QW], F32, tag="mm", name="rb")
            nc.tensor.matmul(
                rb, lhsT=onesb[r0:r0 + 1, :],
                rhs=recip_bf[r0:r0 + 1, h2, :], start=True, stop=True,
                tile_position=(r0, 0))
            sl = attnT[po:po + HD, s, ts(qs, QW)]
            nc.vector.tensor_mul(out=sl, in0=sl, in1=rb[po:po + HD, :])

    def norm_items(s, sums_sb):
        items = []
        box = {}

        def rcp(sums_sb=sums_sb, box=box):
            box["rb"] = recip_bf16(sums_sb)

        items.append(rcp)
        for qs in range(NTS):
            items.append(lambda s=s, qs=qs, box=box: norm_qs(s, qs, box["rb"]))
        return items

    # ---- per-qs normalization + O chunk + out-DMA for the LAST strip ----
    def norm3_items(s, qs, sums_sb):
        box = {}

        def rcp(sums_sb=sums_sb, box=box):
            box["rb"] = recip_bf16(sums_sb)

        def fin(s=s, qs=qs, box=box):
            norm_qs(s, qs, box["rb"])
            for tt in range(4 * qs, 4 * qs + 4):
                for half in range(2):
                    pso = p_mm.tile([P, QW], F32, tag="mm", name="pso3")
                    for c in (2, 3):
                        nc.tensor.matmul(
                            pso, lhsT=attnT[:, c, ts(tt, P)],
                            rhs=wot[:, c, ds(half * QW, QW)],
                            start=(c == 2), stop=(c == 3))
                    dst = osb[:, tt, ds(half * QW, QW)]
                    nc.vector.tensor_add(out=dst, in0=pso, in1=dst)
                nc.sync.dma_start(out=out[ts(tt, P), :], in_=osb[:, tt, :])

        return [rcp, fin]

    # ---- attention: strip s = heads (2s, 2s+1) ----
    def attn_strip(s, qts, kts, sums_sb):
        """Emit attention for both heads of strip s, unit-by-unit.

        A unit covers 2 key blocks for BOTH heads: 4 row-tiled S^T matmuls
        (T0/T8 interleaved -> concurrent), 2 exp activations, then (with
        one unit of lookahead) 4 PV matmuls. diagA/diagB replicate the
        baseline's packed shrinking-width diagonal handling per head.
        """
        hs = (2 * s, 2 * s + 1)

        def emit_s(qs, unit):
            kind, ip = unit
            nfull = NQB * qs
            q0 = qs * QW
            psts = []
            pts = []
            if kind == "full":
                for h2 in range(2):
                    psts.append(p_st.tile(
                        [P, 2, QW], F32, tag=f"st{h2}", name=f"pst{h2}"))
                order = ([(h2, j2) for h2 in range(2) for j2 in range(2)]
                         if SEQ_HEADS else
                         [(h2, j2) for j2 in range(2) for h2 in range(2)])
                for h2, j2 in order:
                    kc = 2 * ip + j2
                    po = h2 * HD
                    nc.tensor.matmul(
                        psts[h2][:, j2, :],
                        lhsT=kts[po:po + HD, ts(kc, P)],
                        rhs=qts[po:po + HD, ts(qs, QW)],
                        start=True, stop=True)
                for h2 in range(2):
                    pt = p_pt.tile([P, 2, QW], BF16, tag="pt", name="pt")
                    nc.scalar.activation(out=pt, in_=psts[h2], func=EXP, scale=0.125)
                    pts.append(pt)
                return pts
            if kind == "diagA":
                # j=0: kc=nfull,   queries [0:512), tri mask on cols 0:128
                # j=1: kc=nfull+1, queries [128:512), tri mask on cols 0:128
                for h2 in range(2):
                    psts.append(p_st.tile(
                        [P, 2, QW], F32, tag=f"st{h2}", name=f"pst{h2}"))
                for h2 in range(2):
                    po = h2 * HD
                    nc.tensor.matmul(
                        psts[h2][:, 0, :],
                        lhsT=kts[po:po + HD, ts(nfull, P)],
                        rhs=qts[po:po + HD, ts(qs, QW)],
                        start=True, stop=True)
                for h2 in range(2):
                    po = h2 * HD
                    nc.tensor.matmul(
                        psts[h2][:, 1, 0:3 * P],
                        lhsT=kts[po:po + HD, ts(nfull + 1, P)],
                        rhs=qts[po:po + HD, ds(q0 + P, 3 * P)],
                        start=True, stop=True)
                for h2 in range(2):
                    pt = p_pt.tile([P, 2, QW], BF16, tag="pt", name="pt")
                    nc.scalar.activation(out=pt, in_=psts[h2], func=EXP, scale=0.125)
                    nc.vector.tensor_mul(pt[:, :, 0:P], pt[:, :, 0:P], maskt2)
                    pts.append(pt)
                return pts
            # diagB: j=2: kc=nfull+2, queries [256:512) at cols 0:256;
            #        j=3: kc=nfull+3, queries [384:512) at cols 256:384
            for h2 in range(2):
                psts.append(p_st.tile([P, QW], F32, tag=f"st{h2}", name=f"pst{h2}"))
            for h2 in range(2):
                po = h2 * HD
                nc.tensor.matmul(
                    psts[h2][:, 0:2 * P],
                    lhsT=kts[po:po + HD, ts(nfull + 2, P)],
                    rhs=qts[po:po + HD, ds(q0 + 2 * P, 2 * P)],
                    start=True, stop=True)
            for h2 in range(2):
                po = h2 * HD
                nc.tensor.matmul(
                    psts[h2][:, 2 * P:3 * P],
                    lhsT=kts[po:po + HD, ts(nfull + 3, P)],
                    rhs=qts[po:po + HD, ds(q0 + 3 * P, P)],
                    start=True, stop=True)
            for h2 in range(2):
                pt = p_pt.tile([P, QW], BF16, tag="pt", name="pt")
                nc.scalar.activation(
                    out=pt[:, 0:3 * P], in_=psts[h2][:, 0:3 * P],
                    func=EXP, scale=0.125)
                # masked regions are cols 0:P and 2P:3P -> one strided mul
                ptr = pt.rearrange("p (a b) -> p a b", b=2 * P)
                nc.vector.tensor_mul(ptr[:, :, 0:P], ptr[:, :, 0:P], maskt2)
                pts.append(pt)
            return pts

        def emit_pv(qs, unit, pts, pvps):
            kind, ip = unit
            nfull = NQB * qs
            if kind == "full":
                for j2 in range(2):
                    kc = 2 * ip + j2
                    for h2 in range(2):
                        nc.tensor.matmul(
                            pvps[h2], lhsT=vplus[:, kc, hs[h2], :],
                            rhs=pts[h2][:, j2, :],
                            start=(kc == 0), stop=False)
            elif kind == "diagA":
                for h2 in range(2):
                    nc.tensor.matmul(
                        pvps[h2], lhsT=vplus[:, nfull, hs[h2], :],
                        rhs=pts[h2][:, 0, :],
                        start=(nfull == 0), stop=False)
                for h2 in range(2):
                    nc.tensor.matmul(
                        pvps[h2][:, P:4 * P], lhsT=vplus[:, nfull + 1, hs[h2], :],
                        rhs=pts[h2][:, 1, 0:3 * P], start=False, stop=False)
            else:
                for h2 in range(2):
                    nc.tensor.matmul(
                        pvps[h2][:, 2 * P:4 * P],
                        lhsT=vplus[:, nfull + 2, hs[h2], :],
                        rhs=pts[h2][:, 0:2 * P], start=False, stop=False)
                for h2 in range(2):
                    nc.tensor.matmul(
                        pvps[h2][:, 3 * P:4 * P],
                        lhsT=vplus[:, nfull + 3, hs[h2], :],
                        rhs=pts[h2][:, 2 * P:3 * P], start=False, stop=True)

        def evac(qs, pvps):
            for h2 in range(2):
                po = h2 * HD
                nc.vector.tensor_copy(
                    out=sums_sb[32 * qs:32 * qs + 1, h2, :],
                    in_=pvps[h2][HD:HD + 1, :])
                if h2 == 0:
                    # same-base copy can ride the Scalar engine (DVE relief)
                    nc.scalar.activation(
                        out=attnT[po:po + HD, s, ts(qs, QW)],
                        in_=pvps[h2][0:HD, :],
                        func=mybir.ActivationFunctionType.Identity)
                else:
                    nc.vector.tensor_copy(
                        out=attnT[po:po + HD, s, ts(qs, QW)],
                        in_=pvps[h2][0:HD, :])
            if s == NSTRIP - 1:
                filler_q.extend(norm3_items(s, qs, sums_sb))

        # flat unit list across qs; the one-unit PV lookahead and the qs
        # eviction run inside the NEXT unit's window (carried across qs and
        # strip boundaries via `pending`) so the exp metronome never stalls
        pump_n = 4 if s == 0 else 2
        pvps = None
        for qs in range(NTS):
            units = [("full", ip) for ip in range(NQB * qs // 2)]
            units.append(("diagA", None))
            units.append(("diagB", None))
            last = len(units) - 1
            for iu, u in enumerate(units):
                pts = emit_s(qs, u)
                if pending[0] is not None:
                    pending[0]()
                    pending[0] = None
                if iu == 0:
                    pvps = [
                        p_pv.tile([HD + 1, QW], F32, tag=f"pv{h2}",
                                  name=f"pvp{h2}")
                        for h2 in range(2)
                    ]
                if iu == last:
                    def fl(qs=qs, u=u, pts=pts, pvps=pvps):
                        emit_pv(qs, u, pts, pvps)
                        evac(qs, pvps)
                    pending[0] = fl
                else:
                    pending[0] = (lambda qs=qs, u=u, pts=pts, pvps=pvps:
                                  emit_pv(qs, u, pts, pvps))
                if not CARRY and iu == last:
                    # flush the PV+evac immediately (no fillers in between)
                    # so the next qs's S matmuls follow as soon as possible
                    pending[0]()
                    pending[0] = None
                    pump(pump_n + 2)
                elif CARRY and iu == last:
                    pass  # hold fillers while the boundary PV is pending
                else:
                    pump(pump_n)
        if CARRY and FLUSH_STRIP_END and pending[0] is not None:
            pending[0]()
            pending[0] = None

    # ================= main schedule =================
    # upfront: only what strip-0 qs=0 needs (Q/K token strip 0, V tiles 0-3);
    # everything else becomes filler work inside the attention stream
    qts = {}
    kts = {}
    sums = {}
    qts[0] = p_qk.tile([P, N], BF16, tag="qt", name="qts")
    kts[0] = p_qk.tile([P, N], BF16, tag="kt", name="kts")
    for f in qk_items(0, wqs0, wks0, qts[0], kts[0])[:4]:
        f()
    for f in v_items(0, 4):
        f()

    for s in range(NSTRIP):
        sums[s] = p_small.tile([P, 2, QW], F32, tag="sums", name="sums_sb")
        nc.gpsimd.memset(sums[s], 1.0)
        if s == 0:
            # rest of strip-0 Q/K, then V jit (one qs ahead of first use)
            filler_q.extend(qk_items(0, wqs0, wks0, qts[0], kts[0], t_lo=1))
            filler_q.extend(v_items(4, 8))
            filler_q.extend(v_items(8, 12))
            filler_q.extend(v_items(12, 16))
        if s + 1 < NSTRIP:
            wqs, wks = load_qk_weights(s + 1)
            qts[s + 1] = p_qk.tile([P, N], BF16, tag="qt", name="qts")
            kts[s + 1] = p_qk.tile([P, N], BF16, tag="kt", name="kts")
            filler_q.extend(qk_items(s + 1, wqs, wks, qts[s + 1], kts[s + 1]))
        attn_strip(s, qts[s], kts[s], sums[s])
        if s < NSTRIP - 1:
            # normalization for this strip runs as fillers inside the next
            # strip's attention
            filler_q.extend(norm_items(s, sums[s]))
        if s < 2:
            filler_q.extend(o_single_items(s))   # O chunk s during strip s+1
    if pending[0] is not None:
        pending[0]()
        pending[0] = None
    drain()
    if dbg is not None:
        nc.sync.dma_start(out=dbg["attnT"], in_=attnT)
        nc.sync.dma_start(out=dbg["vplus"], in_=vplus.rearrange("p a h d -> p (a h d)"))
        for s in range(NSTRIP):
            nc.sync.dma_start(out=dbg["qts"].rearrange("(s p) n -> s p n", p=P)[s],
                              in_=qts[s])
            nc.sync.dma_start(out=dbg["kts"].rearrange("(s p) n -> s p n", p=P)[s],
                              in_=kts[s])


_emit_wrapped = with_exitstack(_emit)

_NC_CACHE = None


def _build():
    global _NC_CACHE
    if _NC_CACHE is not None:
        return _NC_CACHE
    nc = bacc.Bacc("TRN2", target_bir_lowering=False, debug=False)
    xT = nc.dram_tensor("xt", [D, N], BF16, kind="ExternalInput").ap()
    wq = nc.dram_tensor("wq", [D, DC], BF16, kind="ExternalInput").ap()
    wk = nc.dram_tensor("wk", [D, DC], BF16, kind="ExternalInput").ap()
    wv = nc.dram_tensor("wv", [D, DC], BF16, kind="ExternalInput").ap()
    wo = nc.dram_tensor("wo", [DC, D], BF16, kind="ExternalInput").ap()
    bq = nc.dram_tensor("bq", [DC], F32, kind="ExternalInput").ap()
    bk = nc.dram_tensor("bk", [DC], F32, kind="ExternalInput").ap()
    bv = nc.dram_tensor("bv", [DC], F32, kind="ExternalInput").ap()
    masks = nc.dram_tensor("masks", [P, P], BF16, kind="ExternalInput").ap()
    out = nc.dram_tensor("out", [N, D], BF16, kind="ExternalOutput").ap()
    dbg = None
    if DEBUG_DUMP:
        dbg = {
            "attnT": nc.dram_tensor(
                "dbg_attnT", [P, NSTRIP, N], BF16, kind="ExternalOutput").ap(),
            "vplus": nc.dram_tensor(
                "dbg_vplus", [P, NTT * HPC * (HD + 1)], BF16,
                kind="ExternalOutput").ap(),
            "qts": nc.dram_tensor(
                "dbg_qts", [NSTRIP * P, N], BF16, kind="ExternalOutput").ap(),
            "kts": nc.dram_tensor(
                "dbg_kts", [NSTRIP * P, N], BF16, kind="ExternalOutput").ap(),
        }
    with tile.TileContext(nc) as tc:
        _emit_wrapped(tc, xT, wq, wk, wv, wo, bq, bk, bv, masks, out, dbg=dbg)
    nc.compile()
    _NC_CACHE = nc
    return nc


def _make_masks():
    # triangular 0/1 tile for the diagonal blocks of S^T: key <= query kept
    return np.triu(np.ones((P, P), np.float32)).astype(ml_dtypes.bfloat16)


def _in_maps(x, Wq, bq, Wk, bk, Wv, bv, Wo):
    masks = _make_masks()
    maps = []
    for b in range(B):
        xt_b = np.ascontiguousarray(np.asarray(x[b]).T)
        for g in range(GROUPS):
            sl = slice(g * DC, (g + 1) * DC)
            bf = ml_dtypes.bfloat16
            maps.append({
                "xt": xt_b.astype(bf),
                "wq": np.ascontiguousarray(Wq[:, sl]).astype(bf),
                "wk": np.ascontiguousarray(Wk[:, sl]).astype(bf),
                "wv": np.ascontiguousarray(Wv[:, sl]).astype(bf),
                "wo": np.ascontiguousarray(Wo[sl, :]).astype(bf),
                "bq": np.ascontiguousarray(bq[sl]),
                "bk": np.ascontiguousarray(bk[sl]),
                "bv": np.ascontiguousarray(bv[sl]),
                "masks": masks,
            })
    return maps


def run(inputs, trace=False, tmpdir=None):
    """Build+run on 8 cores. Returns (out [B,N,D] f32, BassKernelResults)."""
    x = np.asarray(inputs["x"], np.float32)
    args = [np.asarray(inputs[k], np.float32) for k in
            ("Wq", "bq", "Wk", "bk", "Wv", "bv", "Wo")]
    bo = np.asarray(inputs["bo"], np.float32)
    nc = _build()
    maps = _in_maps(x, *args)
    if trace:
        bass_utils.upload_artifacts = lambda d: d
    res = bass_utils.run_bass_kernel_spmd(
        nc, maps, core_ids=list(range(8)), trace=trace, tmpdir=tmpdir)
    out = np.empty((B, N, D), np.float32)
    for b in range(B):
        out[b] = (res.results[2 * b]["out"].astype(np.float32)
                  + res.results[2 * b + 1]["out"].astype(np.float32) + bo)
    return out, res


def kernel(**inputs):
    out, _ = run(inputs)
    return out
